# revision 41
# baseline (speedup 1.0000x reference)
"""Trainium2 Bass kernel for the LocalizeModule retrieval problem.

Computation (reference):
    f  = relu(feat @ W1.T + b1) @ W2.T + b2        # [F, H]
    k  = keyword @ Wk.T + bk                       # [K, H]
    out = (cos_sim(k, f) + 1) * 0.49               # [K, F]

Sharding across 8 cores (v12):
  * frames (F) sharded for the MLP: each core projects its F/8 frames
    (layer 1 in fp8 DoubleRow, layer 2 in bf16);
  * keywords (K) sharded for the keyword projection (fp8 DoubleRow on
    K/8 keywords): the raw fp8 projection plus the partition-major
    1/||k|| tile ship in ONE packed uint8 AllGather that fires ~6 us
    after the projection lands, fully overlapped with the frame MLP;
  * score GEMM per core: [K, FS] = k8_all.T @ f8 in fp8 DoubleRow.
    Keyword tiles are stationary, so 1/||k|| is a per-partition ScalarE
    scale and 0.49/||f|| is a partition-broadcast row (all local).

Output is the [K, FS] score shard; the host concatenates along F.
"""

import numpy as np
import ml_dtypes

import concourse.bass as bass  # noqa: F401  (bass types used via tile/bacc)
import concourse.mybir as mybir
import concourse.tile as tile
from concourse import bacc
from concourse.bass_utils import run_bass_kernel_spmd

P = 128
H = 1024
F = 8192
K = 4096
NCORES = 8
FS = F // NCORES          # 1024 frames per core
KS = K // NCORES          # 512 keywords per core
HO = H // P               # 8 partition chunks of the hidden dim
NCH = 512                 # matmul moving/free chunk (one PSUM bank of fp32)
F_CHUNKS = FS // NCH      # 2
K_TILES = K // P          # 32
KS_TILES = KS // P        # 4
EPS = 1e-8
OUT_SCALE = 0.49

BF16 = mybir.dt.bfloat16
FP8 = mybir.dt.float8e4
F32 = mybir.dt.float32
U8 = mybir.dt.uint8
AF = mybir.ActivationFunctionType
ALU = mybir.AluOpType

KLIN_FP8 = True           # keyword projection in fp8 DoubleRow
MLP1_FP8 = True           # frame MLP layer 1 in fp8 DoubleRow
MLP2_FP8 = False          # frame MLP layer 2 in bf16 (fills the AG window)
KLIN_DT = FP8 if KLIN_FP8 else BF16
MLP1_DT = FP8 if MLP1_FP8 else BF16
MLP2_DT = FP8 if MLP2_FP8 else BF16
HT_DT = FP8 if MLP2_FP8 else BF16

_CACHE = {}

LAST_EXEC_NS = None
LAST_RESULTS = None


def _emit(tc, io):
    nc = tc.nc
    featT_d, kwT_d, w1t_d, w2t_d, wkt_d, b1_d, b2_d, bk_d, out_d = io

    import contextlib

    with contextlib.ExitStack() as ctx:
        const = ctx.enter_context(tc.tile_pool(name="const", bufs=1))
        psum = ctx.enter_context(tc.tile_pool(name="psum", bufs=1, space="PSUM"))
        dram = ctx.enter_context(tc.tile_pool(name="dram", bufs=1, space="DRAM"))

        # ---- persistent SBUF tensors -------------------------------------
        wkt_s = const.tile([P, HO, H], KLIN_DT)
        w2t_s = const.tile([P, HO, H], MLP2_DT)
        b1_s = const.tile([P, HO], F32)
        b2_s = const.tile([P, HO], F32)
        bk_s = const.tile([P, HO], F32)
        ones_s = const.tile([P, 1], BF16)
        bias049_s = const.tile([P, 1], F32)
        k8_all = const.tile([P, HO, K], FP8)      # gathered raw keyword proj
        hT_s = const.tile([P, HO, FS], HT_DT)     # relu(W1 @ featT + b1)
        f8_s = const.tile([P, HO, FS], FP8)       # projected frames, fp8
        rkn_p = const.tile([P, K_TILES], F32)     # 1/||k||, partition-major
        rfn_row = const.tile([1, FS], F32)        # 0.49/||f||, row layout
        rfn_b = const.tile([P, FS], F32)          # ... bcast on partitions

        nc.vector.memset(bias049_s[:], OUT_SCALE)
        nc.vector.memset(ones_s[:], 1.0)

        # packed AllGather bounce: [fp8 k8 shard || f32 [P,4] 1/||k|| tile]
        CC_K8 = P * HO * KS
        CC_BYTES = CC_K8 + 2048
        cc_in = dram.tile([CC_BYTES], U8)
        cc_out = dram.tile([NCORES, CC_BYTES], U8, addr_space="Shared")

        DRMODE = mybir.MatmulPerfMode.DoubleRow

        def mm_accum(ps, lhs_t, lhs_sl, rhs_t, rhs_sl, fp8):
            if fp8:
                for i in range(HO // 2):
                    nc.tensor.matmul(
                        ps, lhs_t[:, 2 * i:2 * i + 2, lhs_sl],
                        rhs_t[:, 2 * i:2 * i + 2, rhs_sl],
                        start=(i == 0), stop=(i == HO // 2 - 1),
                        perf_mode=DRMODE,
                    )
            else:
                for ho in range(HO):
                    nc.tensor.matmul(
                        ps, lhs_t[:, ho, lhs_sl], rhs_t[:, ho, rhs_sl],
                        start=(ho == 0), stop=(ho == HO - 1),
                    )

        with tc.tile_pool(name="work", bufs=1) as work:

            def emit_tree_sum(sq, width):
                """Pairwise-tree DVE sum of sq[:, ho, :] over the HO axis."""
                tmps = []
                for i in range(HO // 2):
                    tmp = work.tile([P, width], BF16, tag="tsum", bufs=4, name="tsum")
                    nc.vector.tensor_tensor(
                        tmp[:], sq[:, 2 * i, :width], sq[:, 2 * i + 1, :width], ALU.add
                    )
                    tmps.append(tmp)
                nc.vector.tensor_tensor(tmps[0][:], tmps[0][:], tmps[1][:], ALU.add)
                nc.vector.tensor_tensor(tmps[2][:], tmps[2][:], tmps[3][:], ALU.add)
                ssum = work.tile([P, width], BF16, tag="sqs", bufs=3, name="ssum")
                nc.vector.tensor_tensor(ssum[:], tmps[0][:], tmps[2][:], ALU.add)
                return ssum

            # ---- phase K: keyword projection on this core's K/8 shard ----
            with tc.tile_pool(name="mlp_in", bufs=1) as mlp_in:
                kw_s = mlp_in.tile([P, HO, KS], KLIN_DT)
                sqk_s = mlp_in.tile([P, HO, KS], BF16)
                k8_dir = mlp_in.tile([P, HO, KS], FP8)
                featT_s = mlp_in.tile([P, HO, FS], MLP1_DT)
                w1t_s = mlp_in.tile([P, HO, H], MLP1_DT)
                for ho in range(HO):
                    nc.sync.dma_start(kw_s[:, ho], kwT_d[:, ho])
                    nc.sync.dma_start(wkt_s[:, ho], wkt_d[:, ho])
                nc.sync.dma_start(bk_s[:], bk_d[:])
                nc.sync.dma_start(b1_s[:], b1_d[:])
                nc.sync.dma_start(b2_s[:], b2_d[:])
                # MLP1 inputs queue behind the keyword-projection inputs
                for ho in range(HO):
                    nc.sync.dma_start(w1t_s[:, ho], w1t_d[:, ho])
                    nc.sync.dma_start(featT_s[:, ho, 0:NCH], featT_d[:, ho, 0:NCH])
                nc.sync.dma_start(featT_s[:, :, NCH:FS], featT_d[:, :, NCH:FS])
                nc.sync.dma_start(w2t_s[:], w2t_d[:])

                for mh in range(2):
                    kk_pss = [
                        psum.tile([P, KS], F32, tag="mm", bufs=6, name="kk_ps")
                        for _ in range(HO // 2)
                    ]
                    for hh in range(2):
                        for mi in range(HO // 2):
                            mo = mh * (HO // 2) + mi
                            msl = slice(mo * P, (mo + 1) * P)
                            if KLIN_FP8:
                                for i in range(hh * 2, hh * 2 + 2):
                                    nc.tensor.matmul(
                                        kk_pss[mi][:],
                                        wkt_s[:, 2 * i:2 * i + 2, msl],
                                        kw_s[:, 2 * i:2 * i + 2, :],
                                        start=(i == 0), stop=(i == HO // 2 - 1),
                                        perf_mode=DRMODE,
                                    )
                            else:
                                for ho in range(hh * (HO // 2), (hh + 1) * (HO // 2)):
                                    nc.tensor.matmul(
                                        kk_pss[mi][:],
                                        wkt_s[:, ho, msl],
                                        kw_s[:, ho, :],
                                        start=(ho == 0), stop=(ho == HO - 1),
                                    )
                    for mi in range(HO // 2):
                        mo = mh * (HO // 2) + mi
                        nc.vector.tensor_scalar_add(
                            k8_dir[:, mo, :], kk_pss[mi][:], bk_s[:, mo:mo + 1]
                        )
                        # squares straight from the quantized shard (the tiny
                        # extra norm noise only multiplies cos, which is small)
                        nc.vector.tensor_tensor(
                            sqk_s[:, mo, :], k8_dir[:, mo, :], k8_dir[:, mo, :],
                            ALU.mult,
                        )

                # partition-major 1/||k|| for this shard: [P, KS_TILES] f32
                ssum_k = emit_tree_sum(sqk_s, KS)
                rkn4 = work.tile([P, KS_TILES], F32, tag="rkn4", bufs=1, name="rkn4")
                for sub in range(KS_TILES):
                    nkp_ps = psum.tile([P, 1], F32, tag="cn", bufs=2, name="nkp_ps")
                    nc.tensor.matmul(
                        nkp_ps[:], ssum_k[:, sub * P:(sub + 1) * P], ones_s[:],
                        start=True, stop=True,
                    )
                    nc.scalar.copy(rkn4[:, sub:sub + 1], nkp_ps[:])
                nc.scalar.sqrt(rkn4[:], rkn4[:])
                nc.vector.tensor_scalar_max(rkn4[:], rkn4[:], EPS)
                nc.vector.reciprocal(rkn4[:], rkn4[:])

                # one packed AllGather: fp8 shard + bitcast f32 norm tile
                nc.scalar.dma_start(cc_in[0:CC_K8], k8_dir[:].bitcast(U8))
                nc.sync.dma_start(
                    cc_in[CC_K8:CC_BYTES], rkn4[:].bitcast(U8)
                )
                nc.gpsimd.collective_compute(
                    "AllGather",
                    mybir.AluOpType.bypass,
                    replica_groups=[list(range(NCORES))],
                    ins=[cc_in.opt()],
                    outs=[cc_out.opt()],
                )
                # rank-0's keyword chunk gates the first score matmul, so
                # the k8 readbacks for the first ranks go out first; the tiny
                # norm tiles follow (only the epilogue needs them)
                for r in range(NCORES // 2):
                    nc.sync.dma_start(
                        k8_all[:, :, r * KS:(r + 1) * KS].bitcast(U8),
                        cc_out[r, 0:CC_K8],
                    )
                for r in range(NCORES):
                    nc.sync.dma_start(
                        rkn_p[:, r * KS_TILES:(r + 1) * KS_TILES].bitcast(U8),
                        cc_out[r, CC_K8:CC_BYTES],
                    )
                for r in range(NCORES // 2, NCORES):
                    nc.sync.dma_start(
                        k8_all[:, :, r * KS:(r + 1) * KS].bitcast(U8),
                        cc_out[r, 0:CC_K8],
                    )

                # ---- MLP layer 1 ------------------------------------------
                for c in range(F_CHUNKS):
                    for mo in range(HO):
                        h1_ps = psum.tile([P, NCH], F32, tag="mm", bufs=6, name="h1_ps")
                        mm_accum(h1_ps[:], w1t_s, slice(mo * P, (mo + 1) * P),
                                 featT_s, slice(c * NCH, (c + 1) * NCH), MLP1_FP8)
                        nc.scalar.activation(
                            hT_s[:, mo, c * NCH:(c + 1) * NCH],
                            h1_ps[:],
                            AF.Relu,
                            bias=b1_s[:, mo:mo + 1],
                            scale=1.0,
                        )

            # ---- MLP layer 2 + frame norms (row layout) ------------------
            for c in range(F_CHUNKS):
                csl = slice(c * NCH, (c + 1) * NCH)
                sqf = work.tile([P, HO, NCH], BF16, tag="sqf", bufs=2, name="sqf")
                for mo in range(HO):
                    f2_ps = psum.tile([P, NCH], F32, tag="mm", bufs=6, name="f2_ps")
                    mm_accum(f2_ps[:], w2t_s, slice(mo * P, (mo + 1) * P),
                             hT_s, csl, MLP2_FP8)
                    nc.vector.tensor_scalar_add(
                        f8_s[:, mo, csl], f2_ps[:], b2_s[:, mo:mo + 1],
                    )
                    nc.scalar.activation(
                        sqf[:, mo, :], f2_ps[:], AF.Square,
                        bias=b2_s[:, mo:mo + 1], scale=1.0,
                    )
                # frame norms accumulate on the PE; per-chunk reciprocal +
                # broadcast so chunk 0's row is ready before the score starts
                nf_ps = psum.tile([1, NCH], F32, tag="cn", bufs=2, name="nf_ps")
                for mo in range(HO):
                    nc.tensor.matmul(nf_ps[:], ones_s[:], sqf[:, mo, :],
                                     start=(mo == 0), stop=(mo == HO - 1))
                rf = rfn_row[0:1, csl]
                nc.scalar.activation(
                    rf, nf_ps[:], AF.Sqrt, bias=0.0,
                    scale=1.0 / (OUT_SCALE * OUT_SCALE),
                )
                nc.vector.tensor_scalar_max(rf, rf, EPS / OUT_SCALE)
                nc.vector.reciprocal(rf, rf)
                nc.gpsimd.partition_broadcast(rfn_b[:, csl], rf)

            # ---- score GEMM (fp8 DoubleRow, keyword tiles stationary) ----
            for kt in range(K_TILES):
                ksl = slice(kt * P, (kt + 1) * P)
                s_pss = [
                    psum.tile([P, NCH], F32, tag="mm", bufs=6, name="s_ps")
                    for _ in range(F_CHUNKS)
                ]
                for s in range(HO // 2):
                    lhs = k8_all[:, 2 * s:2 * s + 2, ksl]
                    for c in range(F_CHUNKS):
                        rhs = f8_s[:, 2 * s:2 * s + 2, c * NCH:(c + 1) * NCH]
                        nc.tensor.matmul(
                            s_pss[c][:], lhs, rhs,
                            start=(s == 0), stop=(s == HO // 2 - 1),
                            perf_mode=DRMODE,
                        )
                stage = work.tile([P, FS], F32, tag="out_t", bufs=4, name="stage")
                for c in range(F_CHUNKS):
                    csl = slice(c * NCH, (c + 1) * NCH)
                    tmp = work.tile([P, NCH], F32, tag="tmp", bufs=4, name="tmp")
                    nc.vector.tensor_tensor(
                        tmp[:], s_pss[c][:], rfn_b[:, csl], ALU.mult,
                    )
                    if c == 0:
                        nc.scalar.activation(
                            stage[:, csl], tmp[:], AF.Identity,
                            bias=bias049_s[:, 0:1], scale=rkn_p[:, kt:kt + 1],
                        )
                    else:
                        nc.vector.tensor_scalar(
                            stage[:, csl], tmp[:],
                            rkn_p[:, kt:kt + 1], OUT_SCALE,
                            ALU.mult, ALU.add,
                        )
                nc.sync.dma_start(out_d[ksl, :], stage[:])


def build():
    """Build + compile the (core-agnostic) Bass program once."""
    key = "nc_v14" + str((KLIN_FP8, MLP1_FP8, MLP2_FP8))
    if key in _CACHE:
        return _CACHE[key]
    nc = bacc.Bacc(
        "TRN2",
        target_bir_lowering=False,
        debug=False,
        enable_asserts=False,
        num_devices=NCORES,
    )
    featT_d = nc.dram_tensor("featT", [P, HO, FS], MLP1_DT, kind="ExternalInput").ap()
    kwT_d = nc.dram_tensor("kwT", [P, HO, KS], KLIN_DT, kind="ExternalInput").ap()
    w1t_d = nc.dram_tensor("w1t", [P, HO, H], MLP1_DT, kind="ExternalInput").ap()
    w2t_d = nc.dram_tensor("w2t", [P, HO, H], MLP2_DT, kind="ExternalInput").ap()
    wkt_d = nc.dram_tensor("wkt", [P, HO, H], KLIN_DT, kind="ExternalInput").ap()
    b1_d = nc.dram_tensor("b1t", [P, HO], F32, kind="ExternalInput").ap()
    b2_d = nc.dram_tensor("b2t", [P, HO], F32, kind="ExternalInput").ap()
    bk_d = nc.dram_tensor("bkt", [P, HO], F32, kind="ExternalInput").ap()
    out_d = nc.dram_tensor("out", [K, FS], F32, kind="ExternalOutput").ap()

    io = (featT_d, kwT_d, w1t_d, w2t_d, wkt_d, b1_d, b2_d, bk_d, out_d)
    with tile.TileContext(nc) as tc:
        _emit(tc, io)
    nc.compile()
    _CACHE[key] = nc
    return nc


def _part_tile(a):
    """[D0, rest...] with D0 = o*P + p  ->  [P, D0//P, rest...]"""
    d0 = a.shape[0]
    return np.ascontiguousarray(
        a.reshape(d0 // P, P, *a.shape[1:]).swapaxes(0, 1)
    )


def make_in_maps(feat, keyword, W1, b1, W2, b2, Wk, bk):
    bf = ml_dtypes.bfloat16
    f8 = ml_dtypes.float8_e4m3
    t_klin = f8 if KLIN_FP8 else bf
    t_mlp1 = f8 if MLP1_FP8 else bf
    t_mlp2 = f8 if MLP2_FP8 else bf
    feat = np.asarray(feat, np.float32)
    keyword = np.asarray(keyword, np.float32)
    w1t = _part_tile(np.ascontiguousarray(np.asarray(W1, np.float32).T)).astype(t_mlp1)
    w2t = _part_tile(np.ascontiguousarray(np.asarray(W2, np.float32).T)).astype(t_mlp2)
    wkt = _part_tile(np.ascontiguousarray(np.asarray(Wk, np.float32).T)).astype(t_klin)
    b1t = _part_tile(np.asarray(b1, np.float32))                        # [P, HO]
    b2t = _part_tile(np.asarray(b2, np.float32))
    bkt = _part_tile(np.asarray(bk, np.float32))

    in_maps = []
    for c in range(NCORES):
        featT_c = _part_tile(
            np.ascontiguousarray(feat[c * FS:(c + 1) * FS, :].T)
        ).astype(t_mlp1)                                                # [P, HO, FS]
        kwT_c = _part_tile(
            np.ascontiguousarray(keyword[c * KS:(c + 1) * KS, :].T)
        ).astype(t_klin)                                                # [P, HO, KS]
        in_maps.append({
            "featT": featT_c,
            "kwT": kwT_c,
            "w1t": w1t,
            "w2t": w2t,
            "wkt": wkt,
            "b1t": b1t,
            "b2t": b2t,
            "bkt": bkt,
        })
    return in_maps


def assemble_out(shards):
    """shards[c] is the [K, FS] score tile for frames of core c."""
    return np.ascontiguousarray(
        np.concatenate([np.asarray(s) for s in shards], axis=1)
    ).astype(np.float32)


def kernel(feat, keyword, W1, b1, W2, b2, Wk, bk, _trace=False):
    global LAST_EXEC_NS, LAST_RESULTS
    nc = build()
    in_maps = make_in_maps(feat, keyword, W1, b1, W2, b2, Wk, bk)
    res = run_bass_kernel_spmd(
        nc,
        in_maps,
        core_ids=list(range(NCORES)),
        trace=_trace,
    )
    LAST_EXEC_NS = res.exec_time_ns
    LAST_RESULTS = res
    return assemble_out([res.results[c]["out"] for c in range(NCORES)])


# revision 42
# speedup vs baseline: 1.0083x; 1.0083x over previous
"""Trainium2 Bass kernel for the LocalizeModule retrieval problem.

Computation (reference):
    f  = relu(feat @ W1.T + b1) @ W2.T + b2        # [F, H]
    k  = keyword @ Wk.T + bk                       # [K, H]
    out = (cos_sim(k, f) + 1) * 0.49               # [K, F]

Sharding across 8 cores (v12):
  * frames (F) sharded for the MLP: each core projects its F/8 frames
    (layer 1 in fp8 DoubleRow, layer 2 in bf16);
  * keywords (K) sharded for the keyword projection (fp8 DoubleRow on
    K/8 keywords): the raw fp8 projection plus the partition-major
    1/||k|| tile ship in ONE packed uint8 AllGather that fires ~6 us
    after the projection lands, fully overlapped with the frame MLP;
  * score GEMM per core: [K, FS] = k8_all.T @ f8 in fp8 DoubleRow.
    Keyword tiles are stationary, so 1/||k|| is a per-partition ScalarE
    scale and 0.49/||f|| is a partition-broadcast row (all local).

Output is the [K, FS] score shard; the host concatenates along F.
"""

import numpy as np
import ml_dtypes

import concourse.bass as bass  # noqa: F401  (bass types used via tile/bacc)
import concourse.mybir as mybir
import concourse.tile as tile
from concourse import bacc
from concourse.bass_utils import run_bass_kernel_spmd

P = 128
H = 1024
F = 8192
K = 4096
NCORES = 8
FS = F // NCORES          # 1024 frames per core
KS = K // NCORES          # 512 keywords per core
HO = H // P               # 8 partition chunks of the hidden dim
NCH = 512                 # matmul moving/free chunk (one PSUM bank of fp32)
F_CHUNKS = FS // NCH      # 2
K_TILES = K // P          # 32
KS_TILES = KS // P        # 4
EPS = 1e-8
OUT_SCALE = 0.49

BF16 = mybir.dt.bfloat16
FP8 = mybir.dt.float8e4
F32 = mybir.dt.float32
U8 = mybir.dt.uint8
AF = mybir.ActivationFunctionType
ALU = mybir.AluOpType

KLIN_FP8 = True           # keyword projection in fp8 DoubleRow
MLP1_FP8 = True           # frame MLP layer 1 in fp8 DoubleRow
MLP2_FP8 = False          # frame MLP layer 2 in bf16 (fills the AG window)
KLIN_DT = FP8 if KLIN_FP8 else BF16
MLP1_DT = FP8 if MLP1_FP8 else BF16
MLP2_DT = FP8 if MLP2_FP8 else BF16
HT_DT = FP8 if MLP2_FP8 else BF16

_CACHE = {}

LAST_EXEC_NS = None
LAST_RESULTS = None


def _emit(tc, io):
    nc = tc.nc
    featT_d, kwT_d, w1t_d, w2t_d, wkt_d, b1_d, b2_d, bk_d, out_d = io

    import contextlib

    with contextlib.ExitStack() as ctx:
        const = ctx.enter_context(tc.tile_pool(name="const", bufs=1))
        psum = ctx.enter_context(tc.tile_pool(name="psum", bufs=1, space="PSUM"))
        dram = ctx.enter_context(tc.tile_pool(name="dram", bufs=1, space="DRAM"))

        # ---- persistent SBUF tensors -------------------------------------
        wkt_s = const.tile([P, HO, H], KLIN_DT)
        w2t_s = const.tile([P, HO, H], MLP2_DT)
        b1_s = const.tile([P, HO], F32)
        b2_s = const.tile([P, HO], F32)
        bk_s = const.tile([P, HO], F32)
        ones_s = const.tile([P, 1], BF16)
        bias049_s = const.tile([P, 1], F32)
        k8_all = const.tile([P, HO, K], FP8)      # gathered raw keyword proj
        hT_s = const.tile([P, HO, FS], HT_DT)     # relu(W1 @ featT + b1)
        f8_s = const.tile([P, HO, FS], FP8)       # projected frames, fp8
        rkn_p = const.tile([P, K_TILES], F32)     # 1/||k||, partition-major
        rfn_row = const.tile([1, FS], F32)        # 0.49/||f||, row layout
        rfn_b = const.tile([P, FS], F32)          # ... bcast on partitions

        nc.vector.memset(bias049_s[:], OUT_SCALE)
        nc.vector.memset(ones_s[:], 1.0)

        # packed AllGather bounce: [fp8 k8 shard || f32 [P,4] 1/||k|| tile]
        CC_K8 = P * HO * KS
        CC_BYTES = CC_K8 + 2048
        cc_in = dram.tile([CC_BYTES], U8)
        cc_out = dram.tile([NCORES, CC_BYTES], U8, addr_space="Shared")

        DRMODE = mybir.MatmulPerfMode.DoubleRow

        def mm_accum(ps, lhs_t, lhs_sl, rhs_t, rhs_sl, fp8):
            if fp8:
                for i in range(HO // 2):
                    nc.tensor.matmul(
                        ps, lhs_t[:, 2 * i:2 * i + 2, lhs_sl],
                        rhs_t[:, 2 * i:2 * i + 2, rhs_sl],
                        start=(i == 0), stop=(i == HO // 2 - 1),
                        perf_mode=DRMODE,
                    )
            else:
                for ho in range(HO):
                    nc.tensor.matmul(
                        ps, lhs_t[:, ho, lhs_sl], rhs_t[:, ho, rhs_sl],
                        start=(ho == 0), stop=(ho == HO - 1),
                    )

        with tc.tile_pool(name="work", bufs=1) as work:

            def emit_tree_sum(sq, width):
                """Pairwise-tree DVE sum of sq[:, ho, :] over the HO axis."""
                tmps = []
                for i in range(HO // 2):
                    tmp = work.tile([P, width], BF16, tag="tsum", bufs=4, name="tsum")
                    nc.vector.tensor_tensor(
                        tmp[:], sq[:, 2 * i, :width], sq[:, 2 * i + 1, :width], ALU.add
                    )
                    tmps.append(tmp)
                nc.vector.tensor_tensor(tmps[0][:], tmps[0][:], tmps[1][:], ALU.add)
                nc.vector.tensor_tensor(tmps[2][:], tmps[2][:], tmps[3][:], ALU.add)
                ssum = work.tile([P, width], BF16, tag="sqs", bufs=3, name="ssum")
                nc.vector.tensor_tensor(ssum[:], tmps[0][:], tmps[2][:], ALU.add)
                return ssum

            # ---- phase K: keyword projection on this core's K/8 shard ----
            with tc.tile_pool(name="mlp_in", bufs=1) as mlp_in:
                kw_s = mlp_in.tile([P, HO, KS], KLIN_DT)
                sqk_s = mlp_in.tile([P, HO, KS], BF16)
                k8_dir = mlp_in.tile([P, HO, KS], FP8)
                featT_s = mlp_in.tile([P, HO, FS], MLP1_DT)
                w1t_s = mlp_in.tile([P, HO, H], MLP1_DT)
                for ho in range(HO):
                    nc.sync.dma_start(kw_s[:, ho], kwT_d[:, ho])
                    nc.sync.dma_start(wkt_s[:, ho], wkt_d[:, ho])
                nc.sync.dma_start(bk_s[:], bk_d[:])
                nc.sync.dma_start(b1_s[:], b1_d[:])
                nc.sync.dma_start(b2_s[:], b2_d[:])
                # MLP1 inputs queue behind the keyword-projection inputs
                for ho in range(HO):
                    nc.sync.dma_start(w1t_s[:, ho], w1t_d[:, ho])
                    nc.sync.dma_start(featT_s[:, ho, 0:NCH], featT_d[:, ho, 0:NCH])
                nc.sync.dma_start(featT_s[:, :, NCH:FS], featT_d[:, :, NCH:FS])
                nc.sync.dma_start(w2t_s[:], w2t_d[:])

                for mh in range(2):
                    kk_pss = [
                        psum.tile([P, KS], F32, tag="mm", bufs=6, name="kk_ps")
                        for _ in range(HO // 2)
                    ]
                    for hh in range(2):
                        for mi in range(HO // 2):
                            mo = mh * (HO // 2) + mi
                            msl = slice(mo * P, (mo + 1) * P)
                            if KLIN_FP8:
                                for i in range(hh * 2, hh * 2 + 2):
                                    nc.tensor.matmul(
                                        kk_pss[mi][:],
                                        wkt_s[:, 2 * i:2 * i + 2, msl],
                                        kw_s[:, 2 * i:2 * i + 2, :],
                                        start=(i == 0), stop=(i == HO // 2 - 1),
                                        perf_mode=DRMODE,
                                    )
                            else:
                                for ho in range(hh * (HO // 2), (hh + 1) * (HO // 2)):
                                    nc.tensor.matmul(
                                        kk_pss[mi][:],
                                        wkt_s[:, ho, msl],
                                        kw_s[:, ho, :],
                                        start=(ho == 0), stop=(ho == HO - 1),
                                    )
                    for mi in range(HO // 2):
                        mo = mh * (HO // 2) + mi
                        nc.vector.tensor_scalar_add(
                            k8_dir[:, mo, :], kk_pss[mi][:], bk_s[:, mo:mo + 1]
                        )
                        # squares straight from the quantized shard (the tiny
                        # extra norm noise only multiplies cos, which is small)
                        nc.vector.tensor_tensor(
                            sqk_s[:, mo, :], k8_dir[:, mo, :], k8_dir[:, mo, :],
                            ALU.mult,
                        )

                # partition-major 1/||k|| for this shard: [P, KS_TILES] f32
                ssum_k = emit_tree_sum(sqk_s, KS)
                rkn4 = work.tile([P, KS_TILES], F32, tag="rkn4", bufs=1, name="rkn4")
                for sub in range(KS_TILES):
                    nkp_ps = psum.tile([P, 1], F32, tag="cn", bufs=2, name="nkp_ps")
                    nc.tensor.matmul(
                        nkp_ps[:], ssum_k[:, sub * P:(sub + 1) * P], ones_s[:],
                        start=True, stop=True,
                    )
                    nc.scalar.copy(rkn4[:, sub:sub + 1], nkp_ps[:])
                nc.scalar.sqrt(rkn4[:], rkn4[:])
                nc.vector.tensor_scalar_max(rkn4[:], rkn4[:], EPS)
                nc.vector.reciprocal(rkn4[:], rkn4[:])

                # one packed AllGather: fp8 shard + bitcast f32 norm tile
                nc.scalar.dma_start(cc_in[0:CC_K8], k8_dir[:].bitcast(U8))
                nc.sync.dma_start(
                    cc_in[CC_K8:CC_BYTES], rkn4[:].bitcast(U8)
                )
                nc.gpsimd.collective_compute(
                    "AllGather",
                    mybir.AluOpType.bypass,
                    replica_groups=[list(range(NCORES))],
                    ins=[cc_in.opt()],
                    outs=[cc_out.opt()],
                )
                # rank-0's keyword chunk gates the first score matmul, so
                # the k8 readbacks for the first ranks go out first; the tiny
                # norm tiles follow (only the epilogue needs them)
                for r in range(NCORES // 2):
                    nc.sync.dma_start(
                        k8_all[:, :, r * KS:(r + 1) * KS].bitcast(U8),
                        cc_out[r, 0:CC_K8],
                    )
                for r in range(NCORES):
                    nc.sync.dma_start(
                        rkn_p[:, r * KS_TILES:(r + 1) * KS_TILES].bitcast(U8),
                        cc_out[r, CC_K8:CC_BYTES],
                    )
                for r in range(NCORES // 2, NCORES):
                    nc.sync.dma_start(
                        k8_all[:, :, r * KS:(r + 1) * KS].bitcast(U8),
                        cc_out[r, 0:CC_K8],
                    )

                # ---- MLP layer 1 ------------------------------------------
                for c in range(F_CHUNKS):
                    for mo in range(HO):
                        h1_ps = psum.tile([P, NCH], F32, tag="mm", bufs=6, name="h1_ps")
                        mm_accum(h1_ps[:], w1t_s, slice(mo * P, (mo + 1) * P),
                                 featT_s, slice(c * NCH, (c + 1) * NCH), MLP1_FP8)
                        nc.scalar.activation(
                            hT_s[:, mo, c * NCH:(c + 1) * NCH],
                            h1_ps[:],
                            AF.Relu,
                            bias=b1_s[:, mo:mo + 1],
                            scale=1.0,
                        )

            # ---- MLP layer 2 + frame norms (row layout) ------------------
            for c in range(F_CHUNKS):
                csl = slice(c * NCH, (c + 1) * NCH)
                sqf = work.tile([P, HO, NCH], BF16, tag="sqf", bufs=2, name="sqf")
                for mo in range(HO):
                    f2_ps = psum.tile([P, NCH], F32, tag="mm", bufs=6, name="f2_ps")
                    mm_accum(f2_ps[:], w2t_s, slice(mo * P, (mo + 1) * P),
                             hT_s, csl, MLP2_FP8)
                    nc.vector.tensor_scalar_add(
                        f8_s[:, mo, csl], f2_ps[:], b2_s[:, mo:mo + 1],
                    )
                    nc.scalar.activation(
                        sqf[:, mo, :], f2_ps[:], AF.Square,
                        bias=b2_s[:, mo:mo + 1], scale=1.0,
                    )
                # frame norms accumulate on the PE; per-chunk reciprocal +
                # broadcast so chunk 0's row is ready before the score starts
                nf_ps = psum.tile([1, NCH], F32, tag="cn", bufs=2, name="nf_ps")
                for mo in range(HO):
                    nc.tensor.matmul(nf_ps[:], ones_s[:], sqf[:, mo, :],
                                     start=(mo == 0), stop=(mo == HO - 1))
                rf = rfn_row[0:1, csl]
                nc.scalar.activation(
                    rf, nf_ps[:], AF.Sqrt, bias=0.0,
                    scale=1.0 / (OUT_SCALE * OUT_SCALE),
                )
                nc.vector.tensor_scalar_max(rf, rf, EPS / OUT_SCALE)
                nc.vector.reciprocal(rf, rf)
                nc.gpsimd.partition_broadcast(rfn_b[:, csl], rf)

            # ---- score GEMM (fp8 DoubleRow, keyword tiles stationary) ----
            for kt in range(K_TILES):
                ksl = slice(kt * P, (kt + 1) * P)
                s_pss = [
                    psum.tile([P, NCH], F32, tag="mm", bufs=6, name="s_ps")
                    for _ in range(F_CHUNKS)
                ]
                for s in range(HO // 2):
                    lhs = k8_all[:, 2 * s:2 * s + 2, ksl]
                    for c in range(F_CHUNKS):
                        rhs = f8_s[:, 2 * s:2 * s + 2, c * NCH:(c + 1) * NCH]
                        nc.tensor.matmul(
                            s_pss[c][:], lhs, rhs,
                            start=(s == 0), stop=(s == HO // 2 - 1),
                            perf_mode=DRMODE,
                        )
                stage = work.tile([P, FS], F32, tag="out_t", bufs=4, name="stage")
                for c in range(F_CHUNKS):
                    csl = slice(c * NCH, (c + 1) * NCH)
                    tmp = work.tile([P, NCH], F32, tag="tmp", bufs=4, name="tmp")
                    nc.vector.tensor_tensor(
                        tmp[:], s_pss[c][:], rfn_b[:, csl], ALU.mult,
                    )
                    if c == 0:
                        nc.scalar.activation(
                            stage[:, csl], tmp[:], AF.Identity,
                            bias=bias049_s[:, 0:1], scale=rkn_p[:, kt:kt + 1],
                        )
                    else:
                        nc.vector.tensor_scalar(
                            stage[:, csl], tmp[:],
                            rkn_p[:, kt:kt + 1], OUT_SCALE,
                            ALU.mult, ALU.add,
                        )
                if kt % 2 == 0:
                    nc.sync.dma_start(out_d[ksl, :], stage[:])
                else:
                    nc.scalar.dma_start(out_d[ksl, :], stage[:])


def build():
    """Build + compile the (core-agnostic) Bass program once."""
    key = "nc_v15" + str((KLIN_FP8, MLP1_FP8, MLP2_FP8))
    if key in _CACHE:
        return _CACHE[key]
    nc = bacc.Bacc(
        "TRN2",
        target_bir_lowering=False,
        debug=False,
        enable_asserts=False,
        num_devices=NCORES,
    )
    featT_d = nc.dram_tensor("featT", [P, HO, FS], MLP1_DT, kind="ExternalInput").ap()
    kwT_d = nc.dram_tensor("kwT", [P, HO, KS], KLIN_DT, kind="ExternalInput").ap()
    w1t_d = nc.dram_tensor("w1t", [P, HO, H], MLP1_DT, kind="ExternalInput").ap()
    w2t_d = nc.dram_tensor("w2t", [P, HO, H], MLP2_DT, kind="ExternalInput").ap()
    wkt_d = nc.dram_tensor("wkt", [P, HO, H], KLIN_DT, kind="ExternalInput").ap()
    b1_d = nc.dram_tensor("b1t", [P, HO], F32, kind="ExternalInput").ap()
    b2_d = nc.dram_tensor("b2t", [P, HO], F32, kind="ExternalInput").ap()
    bk_d = nc.dram_tensor("bkt", [P, HO], F32, kind="ExternalInput").ap()
    out_d = nc.dram_tensor("out", [K, FS], F32, kind="ExternalOutput").ap()

    io = (featT_d, kwT_d, w1t_d, w2t_d, wkt_d, b1_d, b2_d, bk_d, out_d)
    with tile.TileContext(nc) as tc:
        _emit(tc, io)
    nc.compile()
    _CACHE[key] = nc
    return nc


def _part_tile(a):
    """[D0, rest...] with D0 = o*P + p  ->  [P, D0//P, rest...]"""
    d0 = a.shape[0]
    return np.ascontiguousarray(
        a.reshape(d0 // P, P, *a.shape[1:]).swapaxes(0, 1)
    )


def make_in_maps(feat, keyword, W1, b1, W2, b2, Wk, bk):
    bf = ml_dtypes.bfloat16
    f8 = ml_dtypes.float8_e4m3
    t_klin = f8 if KLIN_FP8 else bf
    t_mlp1 = f8 if MLP1_FP8 else bf
    t_mlp2 = f8 if MLP2_FP8 else bf
    feat = np.asarray(feat, np.float32)
    keyword = np.asarray(keyword, np.float32)
    w1t = _part_tile(np.ascontiguousarray(np.asarray(W1, np.float32).T)).astype(t_mlp1)
    w2t = _part_tile(np.ascontiguousarray(np.asarray(W2, np.float32).T)).astype(t_mlp2)
    wkt = _part_tile(np.ascontiguousarray(np.asarray(Wk, np.float32).T)).astype(t_klin)
    b1t = _part_tile(np.asarray(b1, np.float32))                        # [P, HO]
    b2t = _part_tile(np.asarray(b2, np.float32))
    bkt = _part_tile(np.asarray(bk, np.float32))

    in_maps = []
    for c in range(NCORES):
        featT_c = _part_tile(
            np.ascontiguousarray(feat[c * FS:(c + 1) * FS, :].T)
        ).astype(t_mlp1)                                                # [P, HO, FS]
        kwT_c = _part_tile(
            np.ascontiguousarray(keyword[c * KS:(c + 1) * KS, :].T)
        ).astype(t_klin)                                                # [P, HO, KS]
        in_maps.append({
            "featT": featT_c,
            "kwT": kwT_c,
            "w1t": w1t,
            "w2t": w2t,
            "wkt": wkt,
            "b1t": b1t,
            "b2t": b2t,
            "bkt": bkt,
        })
    return in_maps


def assemble_out(shards):
    """shards[c] is the [K, FS] score tile for frames of core c."""
    return np.ascontiguousarray(
        np.concatenate([np.asarray(s) for s in shards], axis=1)
    ).astype(np.float32)


def kernel(feat, keyword, W1, b1, W2, b2, Wk, bk, _trace=False):
    global LAST_EXEC_NS, LAST_RESULTS
    nc = build()
    in_maps = make_in_maps(feat, keyword, W1, b1, W2, b2, Wk, bk)
    res = run_bass_kernel_spmd(
        nc,
        in_maps,
        core_ids=list(range(NCORES)),
        trace=_trace,
    )
    LAST_EXEC_NS = res.exec_time_ns
    LAST_RESULTS = res
    return assemble_out([res.results[c]["out"] for c in range(NCORES)])
